# revision 48
# baseline (speedup 1.0000x reference)
"""Trainium2 Bass kernel for nn_Attention_Encode (B=4, N=2048, DIM=1024, H=16, DH=64).

Sharding: 16 heads -> 8 cores x 2 heads (tensor parallel). Each core computes
  ztu_g = W_g @ ZT^T          (its 128 output channels = 2 heads)
  attention per (batch, head) with Q=K=V=ztu
  partial_out = ssa_g @ W_g   (row-sharded output projection)
Host sums the 8 partials (the all-reduce step of a row-sharded projection).

On-device layout is fully transposed ("scoresT" = [keys, queries]) so that
softmax needs no transposes: the AV matmul's stationary operand [V | ones]
produces both the numerator and the softmax denominator.
"""
import sys

for _p in ('/opt/trn_rl_repo',):
    if _p not in sys.path:
        sys.path.insert(0, _p)

from contextlib import ExitStack

import numpy as np
import ml_dtypes

import concourse.bacc as bacc
import concourse.mybir as mybir
import concourse.tile as tile
from concourse.bass_utils import run_bass_kernel_spmd
from concourse.masks import make_identity

B, N, C = 4, 2048, 1024          # batch, seq, model dim
KP, DH, HPER = 128, 64, 2        # per-core channels, head dim, heads per core
NQB = 512                        # query block
NKT = 128                        # key tile
NTB = N // NKT                   # 16 key tiles per batch
NTILES = B * NTB                 # 64 n-tiles total
SCALE = DH ** -0.5               # 0.125
BF = mybir.dt.bfloat16
F32 = mybir.dt.float32
F32R = mybir.dt.float32r

_CACHE = {}


def _build_kernel():
    nc = bacc.Bacc("TRN2", target_bir_lowering=False, debug=False)
    ztt = nc.dram_tensor("ztt", [B, C, N], BF, kind="ExternalInput").ap()
    # wgt comes pre-arranged from the host as [p, ci, k] so the DMA is one
    # contiguous transfer (the in-kernel rearrange generated 1024 tiny
    # strided descriptors and took ~12us, gating the first proj1 matmul).
    wgt = nc.dram_tensor("wgt", [KP, 8, KP], BF, kind="ExternalInput").ap()
    wg = nc.dram_tensor("wg", [KP, C], BF, kind="ExternalInput").ap()     # W_g
    out = nc.dram_tensor("out", [B * N, C], BF, kind="ExternalOutput").ap()

    with tile.TileContext(nc) as tc, ExitStack() as ctx:
        _body(ctx, tc, ztt, wgt, wg, out)
    nc.compile()
    return nc


def _body(ctx, tc, ztt, wgt, wg, out):
    nc = tc.nc
    singles = ctx.enter_context(tc.tile_pool(name="singles", bufs=1))
    zin_pool = ctx.enter_context(tc.tile_pool(name="zin", bufs=16))
    sc_pool = ctx.enter_context(tc.tile_pool(name="sc", bufs=2, space="PSUM"))
    av_pool = ctx.enter_context(tc.tile_pool(name="av", bufs=2, space="PSUM"))
    p2_pool = ctx.enter_context(tc.tile_pool(name="p2", bufs=2, space="PSUM"))
    ex_pool = ctx.enter_context(tc.tile_pool(name="ex", bufs=12))
    sn_pool = ctx.enter_context(tc.tile_pool(name="sn", bufs=4))
    rc_pool = ctx.enter_context(tc.tile_pool(name="rc", bufs=4))
    p2s_pool = ctx.enter_context(tc.tile_pool(name="p2s", bufs=8))

    # ---- persistent SBUF ----
    wgt_sb = singles.tile([128, 8, KP], BF)            # [c-in-tile, ci, k]
    nc.sync.dma_start(out=wgt_sb, in_=wgt)
    wg_sb = singles.tile([KP, C], BF)                  # loaded after zin chunk 0
    ident = singles.tile([128, 128], BF)
    make_identity(nc, ident)
    self_f = singles.tile([128, 128], F32)
    nc.vector.memset(self_f, 0.0)
    nc.vector.memset(self_f[0:1, 0:64], 1.0)
    nc.vector.memset(self_f[32:33, 64:128], 1.0)
    sel = singles.tile([128, 128], F32R)               # den -> per-head row broadcast
    nc.vector.tensor_copy(out=sel, in_=self_f)
    dn = singles.tile([128, NQB], F32R)                # dens: head A row 0, head B row 32
    nc.vector.memset(dn[:].bitcast(F32), 0.0)
    # Per-head ztu^T, zero-padded to K=128 so QK matmuls qualify for fast
    # weight load (FWL needs 128 weights) and stay in 128x128 array mode.
    ztuTa = singles.tile([128, B * N], BF)             # head A: rows 64:128 = 0
    ztuTb = singles.tile([128, B * N], BF)             # head B: rows 64:128 = 0
    nc.gpsimd.memset(ztuTa[64:128, :], 0.0)
    nc.gpsimd.memset(ztuTb[64:128, :], 0.0)
    # Combined [A-dims | B-dims] copy of ztu^T, used only by the transposes:
    # one 128-row transpose yields both heads' key tiles at once (64 PE
    # transposes instead of 128).
    ztAll = singles.tile([128, B * N], BF)
    # v-natural per head, padded to M=128: cols [v(64) | ones(1) | 0...]
    ztuN = singles.tile([128, NTILES, 2 * NKT], BF)    # [n-in-tile, nt, head*128+c]
    nc.gpsimd.memset(ztuN, 0.0)
    nc.gpsimd.memset(ztuN[:, :, DH:DH + 1], 1.0)
    nc.gpsimd.memset(ztuN[:, :, NKT + DH:NKT + DH + 1], 1.0)

    # ---- phase 1: proj1 (ztuT = W_g @ ZT^T) + phase 1.5: transposes (ztuN) ----
    def alloc_zin(b):
        return [zin_pool.tile([128, N], BF, tag="zin", name=f"zin{ci}")
                for ci in range(8)]

    def load_zin_chunk(b, zin, jn):
        # One q-block column of all 8 input row-tiles. Issued jn-at-a-time so
        # the first proj1 chunk can start after ~1MB of DMA, and the next
        # batch's loads spread across this batch's attention instead of
        # bursting 4MB into the DMA rings at once (which made the proj2
        # output DMAs queue behind them and stalled the p2s slot recycle).
        for ci in range(8):
            nc.sync.dma_start(
                out=zin[ci][:, jn * NQB:(jn + 1) * NQB],
                in_=ztt[b, ci * 128:(ci + 1) * 128, jn * NQB:(jn + 1) * NQB])

    def proj1_unit(b, zin, jn):
        # One lump: 8 accumulating matmuls + the two ztuT copies. The p1
        # psum tile lives on the p2 tag so the sc pool stays a pure QK/exp
        # double-buffer (V1 had p1 squatting on an sc slot, collapsing the
        # attention pipeline to depth 1 for the duration of each chunk).
        def u():
            p1 = p2_pool.tile([128, NQB], F32, tag="p2", name="p1")
            for ci in range(8):
                nc.tensor.matmul(
                    p1, lhsT=wgt_sb[:, ci, :],
                    rhs=zin[ci][:, jn * NQB:(jn + 1) * NQB],
                    start=(ci == 0), stop=(ci == 7),
                )
            nc.vector.tensor_copy(
                out=ztAll[:, b * N + jn * NQB: b * N + (jn + 1) * NQB],
                in_=p1)
            nc.vector.tensor_copy(
                out=ztuTa[0:64, b * N + jn * NQB: b * N + (jn + 1) * NQB],
                in_=p1[0:64, :])
            nc.vector.tensor_copy(
                out=ztuTb[0:64, b * N + jn * NQB: b * N + (jn + 1) * NQB],
                in_=p1[64:128, :])
        return [u]
    def transpose_units(b, jn):
        # One transpose per key tile covers both heads (pt cols 0-63 = head
        # A's dims, 64-127 = head B's), then one ztuN copy per head.
        def mk(ntl):
            def u():
                nt = b * NTB + ntl
                pt = av_pool.tile([128, NQB], BF, tag="av", name="pt")
                nc.tensor.transpose(
                    pt[:, 0:128],
                    ztAll[:, nt * NKT:(nt + 1) * NKT],
                    ident,
                )
                nc.vector.tensor_copy(
                    out=ztuN[:, nt, 0:DH],
                    in_=pt[:, 0:DH])
                nc.vector.tensor_copy(
                    out=ztuN[:, nt, NKT:NKT + DH],
                    in_=pt[:, DH:2 * DH])
            return u
        return [mk(ntl) for ntl in range(4 * jn, 4 * jn + 4)]

    # ---- phase 2 defs: attention + proj2, software-pipelined across q-blocks ----
    # Emit q-block j's QK/exp/AV before q-block j-1's normalize+proj2 so the
    # PE queue (in-order) never stalls on the DVE normalization chain.
    def attention_block(b, q_abs, nq, units=()):
        # Pipelined within the q-block: group g+1's QK is emitted BEFORE
        # group g's AV, so the in-order PE queue never waits on exp(g) (ACT).
        # Filler work (proj1/proj2/transposes of other blocks) is spread as
        # small units between groups: exp (ACT) is ~2.4x slower than the
        # paired QK, so each group leaves ~0.5us of PE slack that a unit
        # fills. Leftover units drain before the final AV group.
        uit = iter(units)
        nu = len(units)
        avs = [av_pool.tile([128, NQB], F32, tag="av", name=f"av{h}")
               for h in range(HPER)]
        zts = (ztuTa, ztuTb)

        def emit_avs(g, exs):
            for hh in range(HPER):
                for u in range(2):
                    ik = 2 * g + u
                    # 65-wide stationary operand (v + ones col): LDWEIGHTS
                    # time scales with columns, so this halves the AV weight
                    # loads vs the zero-padded 128-wide slice.
                    vT = ztuN[:, b * NTB + ik, hh * NKT: hh * NKT + DH + 1]
                    nc.tensor.matmul(avs[hh][0:DH + 1, 0:nq], lhsT=vT,
                                     rhs=exs[hh][:, u * nq:(u + 1) * nq],
                                     start=(ik == 0), stop=(ik == NTB - 1))

        prev = None
        for g in range(NTB // 2):               # groups of 2 key tiles
            scs, exs = [], []
            for hh in range(HPER):
                sc = sc_pool.tile([128, 2 * NQB], F32, tag="sc",
                                  name=f"sc{hh}")
                qT = zts[hh][:, q_abs:q_abs + nq]
                for u in range(2):
                    ik = 2 * g + u
                    kT = zts[hh][:, b * N + ik * NKT: b * N + (ik + 1) * NKT]
                    nc.tensor.matmul(sc[:, u * nq:(u + 1) * nq],
                                     lhsT=kT, rhs=qT, start=True, stop=True)
                scs.append(sc)
            for hh in range(HPER):
                ex = ex_pool.tile([128, 2 * NQB], BF, tag="ex")
                nc.scalar.activation(
                    out=ex[:, 0:2 * nq], in_=scs[hh][:, 0:2 * nq],
                    func=mybir.ActivationFunctionType.Exp, scale=SCALE)
                exs.append(ex)
            if prev is not None:
                emit_avs(*prev)
            for _ in range((nu + NTB // 2 - 1) // (NTB // 2)):
                un = next(uit, None)
                if un is not None:
                    un()
            prev = (g, exs)
        for un in uit:
            un()
        emit_avs(*prev)
        return avs

    def finish_norm(b, q_abs, nq, avs, tail=False):
        # softmax denominators -> per-head broadcast -> reciprocal -> scale
        nc.vector.tensor_copy(out=dn[0:1, 0:nq], in_=avs[0][DH:DH + 1, 0:nq])
        nc.vector.tensor_copy(out=dn[32:33, 0:nq], in_=avs[1][DH:DH + 1, 0:nq])
        bc = p2_pool.tile([128, NQB], F32, tag="p2", name="bc")
        bcv = bc[:, 0:nq]
        nc.tensor.matmul(bcv, lhsT=sel, rhs=dn[:, 0:nq], start=True, stop=True)
        rc = rc_pool.tile([128, NQB], F32)
        nc.vector.reciprocal_approx_fast(out=rc[:, 0:nq], in_=bcv)
        sn = sn_pool.tile([128, NQB], BF)
        nc.vector.tensor_tensor(
            out=sn[0:64, 0:nq], in0=avs[0][0:DH, 0:nq], in1=rc[0:64, 0:nq],
            op=mybir.AluOpType.mult)
        nc.vector.tensor_tensor(
            out=sn[64:128, 0:nq], in0=avs[1][0:DH, 0:nq],
            in1=rc[64:128, 0:nq],
            op=mybir.AluOpType.mult)
        return sn

    def proj2_units(b, q_abs, nq, sn, split_copies=False):
        # proj2: out[q, :] += ssa_norm_g @ W_g  (both heads contracted).
        # One unit per output tile so they spread across attention groups.
        # split_copies alternates the PSUM->SBUF evacuation between the
        # scalar and vector engines (used for the final q-block, where the
        # copy chain is the kernel tail).
        def mk(t, ch):
            def u():
                p2 = p2_pool.tile([128, NQB], F32, tag="p2", name="p2")
                p2v = p2[:, 0:512]
                nc.tensor.matmul(
                    p2v, lhsT=sn[:, t * 128:(t + 1) * 128],
                    rhs=wg_sb[:, ch * 512:(ch + 1) * 512],
                    start=True, stop=True)
                p2s = p2s_pool.tile([128, 512], BF, tag="p2s", name="p2s")
                if split_copies and (t * 2 + ch) % 2 == 0:
                    nc.scalar.copy(out=p2s, in_=p2v)
                else:
                    nc.vector.tensor_copy(out=p2s, in_=p2v)
                r0 = q_abs + t * 128
                nc.sync.dma_start(
                    out=out[r0:r0 + 128, ch * 512:(ch + 1) * 512], in_=p2s)
            return u
        return [mk(t, ch) for t in range(nq // 128) for ch in range(2)]

    def finish_proj2(b, q_abs, nq, sn):
        for u in proj2_units(b, q_abs, nq, sn, split_copies=True):
            u()

    # ---- main schedule: batch b's proj1/transposes are interleaved into
    # batch b-1's attention at q-block granularity. The previous q-block's
    # norm chain is emitted BEFORE this q-block's QK so its DVE work (which
    # releases the av psum slots) is already done when the AVs need them. ----
    state = {"pending": None, "sn": None}

    def flush_norm(tail=False):
        if state["pending"] is not None:
            b_, qa_, nq_, avs_ = state["pending"]
            state["sn"] = (b_, qa_, nq_, finish_norm(b_, qa_, nq_, avs_,
                                                     tail=tail))
            state["pending"] = None

    def flush_proj2():
        if state["sn"] is not None:
            finish_proj2(*state["sn"])
            state["sn"] = None

    def attention_batch(b, mk_filler=None, split_last=False):
        # split_last (used for the very last batch): the final q-block runs
        # as two 256-wide halves so half 1's norm+proj2 overlap half 2's
        # attention instead of forming the kernel tail.
        segs = [(jq, b * N + jq * NQB, NQB) for jq in range(N // NQB)]
        if split_last:
            jq, qa, _ = segs.pop()
            segs += [(jq, qa, NQB // 2), (None, qa + NQB // 2, NQB // 2)]
        for jq, q_abs, nq in segs:
            flush_norm()
            units = []
            if mk_filler is not None and jq is not None:
                units += mk_filler(jq)
            # proj2 of the previous q-block goes last: its first matmul
            # waits on the norm chain (DVE), so give it a few groups of
            # margin before it enters the in-order PE queue.
            if state["sn"] is not None:
                units += proj2_units(*state["sn"])
                state["sn"] = None
            avs = attention_block(b, q_abs, nq, units)
            state["pending"] = (b, q_abs, nq, avs)

    # PE warm-up spin: ~6us of dependency-free matmuls so the HAM clock gate
    # is already at 8/8 when the first DMA-gated proj1 matmul lands.
    warm = p2_pool.tile([128, NQB], F32, tag="p2", name="warm")
    for _ in range(448):
        nc.tensor.matmul(warm[:, 0:48], lhsT=ident, rhs=ident[:, 0:48],
                         start=True, stop=True)
    del warm

    zs = {0: alloc_zin(0)}
    load_zin_chunk(0, zs[0], 0)
    # wg (proj2 weights, first needed ~40us in) loads after the first
    # proj1 chunk's inputs so it doesn't sit in their DMA critical path.
    nc.sync.dma_start(out=wg_sb, in_=wg)
    for jn in range(1, N // NQB):
        load_zin_chunk(0, zs[0], jn)
    zs[1] = alloc_zin(1)
    for jn in range(N // NQB):
        load_zin_chunk(1, zs[1], jn)
    for b in range(B):
        if b == 0:
            for jn in range(N // NQB):
                for u in proj1_unit(0, zs[0], jn) + transpose_units(0, jn):
                    u()
            zs.pop(0)
        else:
            zin = zs.pop(b)
            if b + 1 < B:
                zs[b + 1] = alloc_zin(b + 1)

            def mk_filler(jq, b=b, zin=zin):
                units = []
                if b + 1 < B:
                    units.append(
                        lambda jq=jq, b=b: load_zin_chunk(b + 1, zs[b + 1], jq))
                units += proj1_unit(b, zin, jq)
                units += transpose_units(b, jq)
                return units

            attention_batch(b - 1, mk_filler)
    attention_batch(B - 1, split_last=True)
    flush_norm(tail=True)
    flush_proj2()


def _get_nc():
    if "nc" not in _CACHE:
        _CACHE["nc"] = _build_kernel()
    return _CACHE["nc"]


def kernel(ZT: np.ndarray, W: np.ndarray) -> np.ndarray:
    ZT = np.asarray(ZT, dtype=np.float32)
    W = np.asarray(W, dtype=np.float32)
    ztt = np.ascontiguousarray(ZT.transpose(0, 2, 1)).astype(ml_dtypes.bfloat16)
    in_maps = []
    for c in range(8):
        wgf = W[c * KP:(c + 1) * KP, :]
        # [p, ci, k]: wgt[p, ci, k] = W_g[k, ci*128+p] (contiguous DMA layout)
        wgt_arr = np.ascontiguousarray(
            wgf.T.reshape(8, KP, KP).transpose(1, 0, 2)
        ).astype(ml_dtypes.bfloat16)
        in_maps.append({
            "ztt": ztt,
            "wgt": wgt_arr,
            "wg": np.ascontiguousarray(wgf).astype(ml_dtypes.bfloat16),
        })
    nc = _get_nc()
    res = run_bass_kernel_spmd(nc, in_maps, core_ids=list(range(8)))
    acc = np.zeros((B * N, C), dtype=np.float32)
    for r in res.results:
        acc += np.asarray(r["out"], dtype=np.float32)
    return acc.reshape(B, N, C)


if __name__ == "__main__":
    rng = np.random.default_rng(0)
    zt = rng.standard_normal((B, N, C), dtype=np.float32)
    w = rng.standard_normal((KP * 8, C), dtype=np.float32) * C ** -0.5
    o = kernel(zt, w)
    print("out", o.shape, o.dtype, float(np.abs(o).mean()))

